# revision 4
# baseline (speedup 1.0000x reference)
"""Trainium2 Bass kernel for nn_DAMPLayer (GNN message passing layer).

Strategy: edges sorted by destination node on host, sharded contiguously by
dst across 8 cores (1250 nodes each) so that all segment reductions are
core-local (no collectives). Per 128-node block, the device gathers
[node_feats | q] rows for edge sources, computes the edge MLP / messages /
logits with feature-major matmuls, performs the edge softmax without max
subtraction (logits are O(1)), and accumulates the weighted message sums via
one-hot matmuls in PSUM. The GRU node update runs feature-major; the elu
offset and message bias are folded into adjusted GRU biases host-side.
"""

import sys

if "/opt/trn_rl_repo" not in sys.path:
    sys.path.insert(0, "/opt/trn_rl_repo")

import numpy as np

import concourse.bass as bass
import concourse.mybir as mybir
import concourse.tile as tile
from concourse import bacc
from concourse.bass_utils import run_bass_kernel_spmd
from concourse.masks import make_identity

F32 = mybir.dt.float32
I16 = mybir.dt.int16
AF = mybir.ActivationFunctionType
OP = mybir.AluOpType

V = 10000          # nodes
M = 640000         # edges
NF = 64            # node feature size
EF = 64            # edge feature size
NH = 128           # node hidden
EH = 128           # edge hidden
CS = 128           # context size
NCORE = 8
NOWN = V // NCORE  # 1250 nodes per core
NBLK = (NOWN + 127) // 128  # 10 blocks per core
LRELU_ALPHA = 0.01


def _roundup(x, m):
    return (x + m - 1) // m * m


def _build_program(bes):
    """Trace + compile the SPMD Bass program for per-block edge counts `bes`."""
    e_pad = sum(bes)
    nc = bacc.Bacc(
        "TRN2",
        target_bir_lowering=False,
        debug=False,
        num_devices=NCORE,
    )

    def din(name, shape, dtype=F32):
        return nc.dram_tensor(name, list(shape), dtype, kind="ExternalInput").ap()

    tbl = din("tbl", [V, 128])                 # [node_feats | q(dev) | pad]
    nfT = din("nfT", [NF, V])
    nf_ownT = din("nf_ownT", [NF, NOWN])
    efT = din("efT", [EF, e_pad])
    gidx = din("gidx", [128, e_pad // 16], I16)
    dstl = din("dstl", [128, e_pad // 128])
    iota = din("iota", [128, 128])
    w_node = din("w_node", [NF, NH])
    b_node = din("b_node", [NH, 1])
    w_edge = din("w_edge", [NF + EF, EH])
    b_edge = din("b_edge", [EH, 1])
    w1 = din("w1", [NH, 1])
    w2 = din("w2", [EH, 1])
    w_msg = din("w_msg", [EH, CS])
    b_msg = din("b_msg", [CS, 1])
    w_ihT = din("w_ihT", [CS, 3 * NH])
    w_hhT = din("w_hhT", [NH, 3 * NH])
    gru_b = din("gru_b", [NH, 4])              # cols: r, z, in, hn
    b_logit_t = din("b_logit", [128, 1])
    out_ap = nc.dram_tensor("h_newT", [128, NOWN], F32, kind="ExternalOutput").ap()

    with tile.TileContext(nc) as tc:
        with tc.tile_pool(name="const", bufs=1) as cp:
            ident = cp.tile([128, 128], F32)
            make_identity(nc, ident[:])
            iota_t = cp.tile([128, 128], F32)
            nc.sync.dma_start(iota_t[:], iota[:])
            wnode_t = cp.tile([NF, NH], F32)
            nc.sync.dma_start(wnode_t[:], w_node[:])
            bnode_t = cp.tile([NH, 1], F32)
            nc.sync.dma_start(bnode_t[:], b_node[:])
            wedge_t = cp.tile([128, EH], F32)
            nc.sync.dma_start(wedge_t[:], w_edge[:])
            bedge_t = cp.tile([EH, 1], F32)
            nc.sync.dma_start(bedge_t[:], b_edge[:])
            w1_t = cp.tile([NH, 1], F32)
            nc.sync.dma_start(w1_t[:], w1[:])
            w2_t = cp.tile([EH, 1], F32)
            nc.sync.dma_start(w2_t[:], w2[:])
            wmsg_t = cp.tile([EH, CS], F32)
            nc.sync.dma_start(wmsg_t[:], w_msg[:])
            bmsg_t = cp.tile([CS, 1], F32)
            nc.sync.dma_start(bmsg_t[:], b_msg[:])
            wihT_t = cp.tile([CS, 3 * NH], F32)
            nc.sync.dma_start(wihT_t[:], w_ihT[:])
            whhT_t = cp.tile([NH, 3 * NH], F32)
            nc.sync.dma_start(whhT_t[:], w_hhT[:])
            grub_t = cp.tile([NH, 4], F32)
            nc.sync.dma_start(grub_t[:], gru_b[:])
            blog_t = cp.tile([128, 1], F32)
            nc.sync.dma_start(blog_t[:], b_logit_t[:])
            ones_t = cp.tile([128, 1], F32)
            nc.vector.memset(ones_t[:], 1.0)
            gidx_t = cp.tile([128, e_pad // 16], I16)
            nc.sync.dma_start(gidx_t[:], gidx[:])
            dstl_t = cp.tile([128, e_pad // 128], F32)
            nc.sync.dma_start(dstl_t[:], dstl[:])
            hv_own = cp.tile([128, NOWN], F32)
            cg_sb = cp.tile([128, NOWN], F32)

            # ---------------- Phase A: q = w1 . h_v for all nodes ----------
            with (
                tc.tile_pool(name="a_sb", bufs=3) as ap_,
                tc.tile_pool(name="a_ps", bufs=2, space="PSUM") as aps,
            ):
                for t in range(0, V, 512):
                    w = min(512, V - t)
                    nft = ap_.tile([NF, 512], F32, tag="nft")
                    nc.sync.dma_start(nft[:, :w], nfT[:, t : t + w])
                    hv_ps = aps.tile([NH, 512], F32, tag="hv_ps")
                    nc.tensor.matmul(
                        hv_ps[:, :w], lhsT=wnode_t[:], rhs=nft[:, :w],
                        start=True, stop=True,
                    )
                    hv_t = ap_.tile([NH, 512], F32, tag="hv_t")
                    nc.scalar.activation(
                        hv_t[:, :w], hv_ps[:, :w], AF.Prelu,
                        bias=bnode_t[:, 0:1], alpha=LRELU_ALPHA,
                    )
                    q_ps = aps.tile([1, 512], F32, tag="q_ps")
                    nc.tensor.matmul(
                        q_ps[0:1, :w], lhsT=w1_t[:, 0:1], rhs=hv_t[:, :w],
                        start=True, stop=True,
                    )
                    q_t = ap_.tile([1, 512], F32, tag="q_t")
                    nc.vector.tensor_copy(q_t[0:1, :w], q_ps[0:1, :w])
                    # scatter q into gather-table column 64 (strided 4B writes)
                    nc.sync.dma_start(
                        out=tbl[t : t + w, 64:65], in_=q_t[0:1, :w]
                    )
                # own-range h_v (feature-major) for the GRU
                for t in range(0, NOWN, 512):
                    w = min(512, NOWN - t)
                    nfo = ap_.tile([NF, 512], F32, tag="nfo")
                    nc.sync.dma_start(nfo[:, :w], nf_ownT[:, t : t + w])
                    hvo_ps = aps.tile([NH, 512], F32, tag="hv_ps")
                    nc.tensor.matmul(
                        hvo_ps[:, :w], lhsT=wnode_t[:], rhs=nfo[:, :w],
                        start=True, stop=True,
                    )
                    nc.scalar.activation(
                        hv_own[:, t : t + w], hvo_ps[:, :w], AF.Prelu,
                        bias=bnode_t[:, 0:1], alpha=LRELU_ALPHA,
                    )

            # ---------------- Phase B: edge blocks -------------------------
            with (
                tc.tile_pool(name="b_big", bufs=2) as bp,
                tc.tile_pool(name="b_sb", bufs=3) as wp,
                tc.tile_pool(name="b_ps", bufs=1, space="PSUM") as ps1,
                tc.tile_pool(name="b_ps2", bufs=2, space="PSUM") as ps2,
            ):
                e_base = 0
                ch_base = 0
                for b in range(NBLK):
                    be = bes[b]
                    nch = be // 128
                    ntl = be // 512
                    nb = min(128, NOWN - b * 128)

                    g_b = bp.tile([128, nch, 128], F32, tag="g_b")
                    nc.gpsimd.dma_gather(
                        g_b[:],
                        tbl[:],
                        gidx_t[:, e_base // 16 : (e_base + be) // 16],
                        be,
                        be,
                        128,
                        single_packet=False,
                    )
                    m_sb = bp.tile([128, nch, 129], F32, tag="m_sb")
                    nc.vector.memset(m_sb[:, 0:nch, 128:129], 1.0)
                    z2_ps = ps1.tile([128, nch], F32, tag="z2_ps")
                    acc_ps = ps1.tile([128, 129], F32, tag="acc_ps")

                    for t in range(ntl):
                        col = t * 512
                        e_in = wp.tile([128, 512], F32, tag="e_in")
                        nc.sync.dma_start(
                            e_in[64:128, :],
                            efT[:, e_base + col : e_base + col + 512],
                        )
                        gt_ps = ps1.tile([64, 512], F32, tag="gt_ps")
                        for c in range(4):
                            ch = t * 4 + c
                            nc.tensor.transpose(
                                gt_ps[0:64, c * 128 : (c + 1) * 128],
                                g_b[:, ch, 0:64],
                                ident[:],
                            )
                        nc.vector.tensor_copy(e_in[0:64, :], gt_ps[0:64, :])
                        he_ps = ps2.tile([128, 512], F32, tag="he_ps")
                        nc.tensor.matmul(
                            he_ps[:], lhsT=wedge_t[:], rhs=e_in[:],
                            start=True, stop=True,
                        )
                        he_sb = wp.tile([128, 512], F32, tag="he_sb")
                        nc.scalar.activation(
                            he_sb[:], he_ps[:], AF.Prelu,
                            bias=bedge_t[:, 0:1], alpha=LRELU_ALPHA,
                        )
                        m_ps = ps2.tile([128, 512], F32, tag="m_ps")
                        for c in range(4):
                            ch = t * 4 + c
                            hc = he_sb[:, c * 128 : (c + 1) * 128]
                            nc.tensor.matmul(
                                m_ps[:, c * 128 : (c + 1) * 128],
                                lhsT=hc, rhs=wmsg_t[:], start=True, stop=True,
                            )
                            nc.tensor.matmul(
                                z2_ps[:, ch : ch + 1],
                                lhsT=hc, rhs=w2_t[:, 0:1],
                                start=True, stop=True,
                            )
                        nc.scalar.copy(
                            m_sb[:, t * 4 : (t + 1) * 4, 0:128],
                            m_ps[:].rearrange("p (c f) -> p c f", c=4),
                        )

                    # block-level logits -> ex
                    qs_v = g_b[:, 0:nch, 64:65].rearrange("p c o -> p (c o)")
                    zt = wp.tile([128, nch], F32, tag="zt")
                    nc.vector.tensor_tensor(zt[:], z2_ps[:], qs_v, op=OP.add)
                    lg = wp.tile([128, nch], F32, tag="lg")
                    nc.scalar.activation(
                        lg[:], zt[:], AF.Prelu, bias=blog_t[:, 0:1],
                        alpha=LRELU_ALPHA,
                    )
                    ex_b = wp.tile([128, nch], F32, tag="ex_b")
                    nc.scalar.activation(ex_b[:], lg[:], AF.Exp)

                    # scatter: acc[v, :] += sum_e onehot(e,v)*ex(e)*[m(e),1]
                    for ch in range(nch):
                        exoh = wp.tile([128, 128], F32, tag="exoh")
                        nc.vector.tensor_scalar(
                            exoh[:],
                            iota_t[:],
                            dstl_t[:, ch_base + ch : ch_base + ch + 1],
                            ex_b[:, ch : ch + 1],
                            OP.is_equal,
                            OP.mult,
                        )
                        nc.tensor.matmul(
                            acc_ps[:],
                            lhsT=exoh[:],
                            rhs=m_sb[:, ch, 0:129],
                            start=(ch == 0),
                            stop=(ch == nch - 1),
                        )

                    # C = acc[:, :128]/den + b_msg ; Cg = elu(C) + 1
                    den = wp.tile([128, 1], F32, tag="den")
                    nc.vector.tensor_scalar(
                        den[:], acc_ps[:, 128:129], 1e-30, None, OP.max
                    )
                    rec = wp.tile([128, 1], F32, tag="rec")
                    nc.vector.reciprocal(rec[:], den[:])
                    c_em = wp.tile([128, 128], F32, tag="c_em")
                    nc.vector.tensor_scalar(
                        c_em[:], acc_ps[:, 0:128], rec[:, 0:1], None, OP.mult
                    )
                    ct_ps = ps1.tile([128, 128], F32, tag="ct_ps")
                    nc.tensor.transpose(ct_ps[:], c_em[:], ident[:])
                    mn_t = wp.tile([128, 128], F32, tag="mn_t")
                    nc.vector.tensor_scalar(
                        mn_t[:], ct_ps[:], bmsg_t[:, 0:1], 0.0, OP.add, OP.min
                    )
                    e1_t = wp.tile([128, 128], F32, tag="e1_t")
                    nc.scalar.activation(e1_t[:], mn_t[:], AF.Exp)
                    mx_t = wp.tile([128, 128], F32, tag="mx_t")
                    nc.vector.tensor_scalar(
                        mx_t[:], ct_ps[:], bmsg_t[:, 0:1], 0.0, OP.add, OP.max
                    )
                    nc.vector.tensor_tensor(
                        cg_sb[:, b * 128 : b * 128 + nb],
                        mx_t[:, 0:nb],
                        e1_t[:, 0:nb],
                        op=OP.add,
                    )
                    e_base += be
                    ch_base += nch

            # ---------------- Phase C: GRU ---------------------------------
            with (
                tc.tile_pool(name="c_sb", bufs=2) as gp,
                tc.tile_pool(name="c_ps", bufs=2, space="PSUM") as cps,
            ):
                for t in range(0, NOWN, 512):
                    w = min(512, NOWN - t)
                    sl = slice(t, t + w)
                    rr_ps = cps.tile([128, 512], F32, tag="rr_ps")
                    nc.tensor.matmul(
                        rr_ps[:, :w], lhsT=wihT_t[:, 0:128], rhs=cg_sb[:, sl],
                        start=True, stop=False,
                    )
                    nc.tensor.matmul(
                        rr_ps[:, :w], lhsT=whhT_t[:, 0:128], rhs=hv_own[:, sl],
                        start=False, stop=True,
                    )
                    zz_ps = cps.tile([128, 512], F32, tag="zz_ps")
                    nc.tensor.matmul(
                        zz_ps[:, :w], lhsT=wihT_t[:, 128:256], rhs=cg_sb[:, sl],
                        start=True, stop=False,
                    )
                    nc.tensor.matmul(
                        zz_ps[:, :w], lhsT=whhT_t[:, 128:256], rhs=hv_own[:, sl],
                        start=False, stop=True,
                    )
                    gin_ps = cps.tile([128, 512], F32, tag="gin_ps")
                    nc.tensor.matmul(
                        gin_ps[:, :w], lhsT=wihT_t[:, 256:384], rhs=cg_sb[:, sl],
                        start=True, stop=True,
                    )
                    ghn_ps = cps.tile([128, 512], F32, tag="ghn_ps")
                    nc.tensor.matmul(
                        ghn_ps[:, :w], lhsT=whhT_t[:, 256:384], rhs=hv_own[:, sl],
                        start=True, stop=True,
                    )
                    r_sb = gp.tile([128, 512], F32, tag="r_sb")
                    nc.scalar.activation(
                        r_sb[:, :w], rr_ps[:, :w], AF.Sigmoid, bias=grub_t[:, 0:1]
                    )
                    z_sb = gp.tile([128, 512], F32, tag="z_sb")
                    nc.scalar.activation(
                        z_sb[:, :w], zz_ps[:, :w], AF.Sigmoid, bias=grub_t[:, 1:2]
                    )
                    t1 = gp.tile([128, 512], F32, tag="t1")
                    nc.vector.tensor_scalar(
                        t1[:, :w], ghn_ps[:, :w], grub_t[:, 3:4], None, OP.add
                    )
                    t2 = gp.tile([128, 512], F32, tag="t2")
                    nc.vector.tensor_tensor(
                        t2[:, :w], t1[:, :w], r_sb[:, :w], op=OP.mult
                    )
                    t3 = gp.tile([128, 512], F32, tag="t3")
                    nc.vector.tensor_tensor(
                        t3[:, :w], t2[:, :w], gin_ps[:, :w], op=OP.add
                    )
                    n_sb = gp.tile([128, 512], F32, tag="n_sb")
                    nc.scalar.activation(
                        n_sb[:, :w], t3[:, :w], AF.Tanh, bias=grub_t[:, 2:3]
                    )
                    u1 = gp.tile([128, 512], F32, tag="u1")
                    nc.vector.tensor_tensor(
                        u1[:, :w], z_sb[:, :w], hv_own[:, sl], op=OP.mult
                    )
                    u2 = gp.tile([128, 512], F32, tag="u2")
                    nc.vector.tensor_tensor(
                        u2[:, :w], z_sb[:, :w], n_sb[:, :w], op=OP.mult
                    )
                    u3 = gp.tile([128, 512], F32, tag="u3")
                    nc.vector.tensor_tensor(
                        u3[:, :w], n_sb[:, :w], u2[:, :w], op=OP.subtract
                    )
                    u4 = gp.tile([128, 512], F32, tag="u4")
                    nc.vector.tensor_tensor(
                        u4[:, :w], u3[:, :w], u1[:, :w], op=OP.add
                    )
                    hn_sb = gp.tile([128, 512], F32, tag="hn_sb")
                    nc.scalar.activation(hn_sb[:, :w], u4[:, :w], AF.Relu)
                    nc.sync.dma_start(out_ap[:, sl], hn_sb[:, :w])

    nc.compile()
    return nc


def _prepare(inputs):
    """Host-side layout prep: sort/shard/pad edges, build per-core input maps."""
    node_feats = np.ascontiguousarray(np.asarray(inputs["node_feats"], np.float32))
    edge_feats = np.ascontiguousarray(np.asarray(inputs["edge_feats"], np.float32))
    src = np.asarray(inputs["src"], np.int32)
    dst = np.asarray(inputs["dst"], np.int32)

    perm = np.argsort(dst, kind="stable")
    dsts = dst[perm]
    srcs = src[perm]

    # block boundaries: per core c, block b covers nodes
    # [c*NOWN + b*128, min(c*NOWN + (b+1)*128, (c+1)*NOWN))
    blk_lo = np.zeros((NCORE, NBLK), np.int64)
    blk_hi = np.zeros((NCORE, NBLK), np.int64)
    for c in range(NCORE):
        for b in range(NBLK):
            lo_node = c * NOWN + b * 128
            hi_node = min(c * NOWN + (b + 1) * 128, (c + 1) * NOWN)
            blk_lo[c, b] = np.searchsorted(dsts, lo_node, side="left")
            blk_hi[c, b] = np.searchsorted(dsts, hi_node, side="left")
    counts = blk_hi - blk_lo
    bes = [int(_roundup(int(counts[:, b].max()), 512)) for b in range(NBLK)]
    e_pad = sum(bes)

    in_maps = []
    shared = None
    for c in range(NCORE):
        srcp = np.zeros(e_pad, np.int32)
        dstl = np.full(e_pad, -1.0, np.float32)
        efp = np.zeros((e_pad, EF), np.float32)
        off = 0
        for b in range(NBLK):
            lo, hi = blk_lo[c, b], blk_hi[c, b]
            n = int(hi - lo)
            sel = perm[lo:hi]
            srcp[off : off + n] = srcs[lo:hi]
            dstl[off : off + n] = (dsts[lo:hi] - (c * NOWN + b * 128)).astype(
                np.float32
            )
            efp[off : off + n] = edge_feats[sel]
            off += bes[b]

        if shared is None:
            w_logit = np.asarray(inputs["W_logit"], np.float32)
            w_ih = np.asarray(inputs["W_ih"], np.float32)
            w_hh = np.asarray(inputs["W_hh"], np.float32)
            b_ih = np.asarray(inputs["b_ih"], np.float32)
            b_hh = np.asarray(inputs["b_hh"], np.float32)
            # elu(x) computed as (max(x,0) + exp(min(x,0))) - 1; the -1 is
            # folded into the GRU input bias: b_ih' = b_ih - W_ih.sum(1)
            b_ih_adj = b_ih - w_ih.sum(axis=1)
            gru_b = np.stack(
                [
                    b_ih_adj[0:128] + b_hh[0:128],
                    b_ih_adj[128:256] + b_hh[128:256],
                    b_ih_adj[256:384],
                    b_hh[256:384],
                ],
                axis=1,
            ).astype(np.float32)
            tbl = np.concatenate(
                [node_feats, np.zeros((V, 128 - NF), np.float32)], axis=1
            )
            shared = {
                "tbl": np.ascontiguousarray(tbl),
                "nfT": np.ascontiguousarray(node_feats.T),
                "iota": np.broadcast_to(
                    np.arange(128, dtype=np.float32), (128, 128)
                ).copy(),
                "w_node": np.ascontiguousarray(np.asarray(inputs["W_node"], np.float32)),
                "b_node": np.asarray(inputs["b_node"], np.float32).reshape(NH, 1),
                "w_edge": np.ascontiguousarray(np.asarray(inputs["W_edge"], np.float32)),
                "b_edge": np.asarray(inputs["b_edge"], np.float32).reshape(EH, 1),
                "w1": np.ascontiguousarray(w_logit[0:NH, 0:1]),
                "w2": np.ascontiguousarray(w_logit[NH : NH + EH, 0:1]),
                "w_msg": np.ascontiguousarray(np.asarray(inputs["W_msg"], np.float32)),
                "b_msg": np.asarray(inputs["b_msg"], np.float32).reshape(CS, 1),
                "w_ihT": np.ascontiguousarray(w_ih.T),
                "w_hhT": np.ascontiguousarray(w_hh.T),
                "gru_b": np.ascontiguousarray(gru_b),
                "b_logit": np.full(
                    (128, 1), float(np.asarray(inputs["b_logit"]).reshape(-1)[0]),
                    np.float32,
                ),
            }

        own = slice(c * NOWN, (c + 1) * NOWN)
        im = dict(shared)
        im["tbl"] = shared["tbl"].copy()  # device writes q into col 64
        im["nf_ownT"] = np.ascontiguousarray(node_feats[own].T)
        im["efT"] = np.ascontiguousarray(efp.T)
        im["gidx"] = np.ascontiguousarray(
            np.tile(srcp.astype(np.int16).reshape(-1, 16).T, (8, 1))
        )
        im["dstl"] = np.ascontiguousarray(dstl.reshape(-1, 128).T)
        in_maps.append(im)

    return bes, in_maps, edge_feats


_CACHE = {}


def kernel(**inputs):
    bes, in_maps, edge_feats = _prepare(inputs)
    key = tuple(bes)
    if key not in _CACHE:
        _CACHE[key] = _build_program(bes)
    nc = _CACHE[key]
    res = run_bass_kernel_spmd(nc, in_maps, core_ids=list(range(NCORE)))
    h_new = np.concatenate(
        [np.asarray(r["h_newT"]).T for r in res.results], axis=0
    )
    return h_new, edge_feats
